# revision 10
# baseline (speedup 1.0000x reference)
"""DANetHead (depth-augmented position attention) Trainium2 kernel.

Sharding: 8 cores = 4 batches x 2 row-halves. Two SPMD launches:
  launch 1: conv5a (3x3, 2048->512) + BN/ReLU, then 1x1 q/k and v^T
  launch 2: depth-augmented attention (fp32 sim + safe softmax), sa,
            conv5c (3x3, 512->512) + BN/ReLU, conv6 (1x1, 512->59)
Host work between launches is layout-only (gather halves, window
slicing, BN folding, tiny dep-derived aug channels).

Matmuls run as float32r (full PE rate at free-dim >= 256; even
innermost counts required, hence 62-wide zero-padded image rows)
except the attention similarity, which is plain fp32 for softmax
stability (sim spans roughly [-1800, 2000]).
"""

import numpy as np
import concourse.bass as bass
import concourse.mybir as mybir
import concourse.tile as tile
from concourse import bass_isa
from concourse.bass_utils import run_bass_kernel_spmd

F32 = mybir.dt.float32
F32R = mybir.dt.float32r
AF = mybir.ActivationFunctionType
EPS = 1e-5

B, CIN, H, W = 4, 2048, 60, 60
WP = 62            # zero-padded row pitch
CI, CQ, COUT = 512, 64, 59
HH = 30            # rows per half
NH = HH * W        # 1800
WIN = 32           # attention row window (30 valid + 2 halo/pad rows)
NI = WIN * W       # 1920
NJC = 29           # j chunks of 128
NJ = NJC * 128     # 3712 padded j
IBW = 480          # attention i-block width (8 rows)
NIB = NI // IBW    # 4
NEG = -1.0e30

TAPS = [(dy, dx) for dy in (-1, 0, 1) for dx in (-1, 0, 1)]


def _split_multiwaits(nc):
    """This walrus build rejects >1 sync-wait per instruction; move the
    extras onto fresh single-wait NOPs inserted just before, same engine."""
    uid = 0
    for bb in nc.m.functions[0].blocks:
        out = []
        for inst in bb.instructions:
            si = inst.sync_info
            if si is not None and si.on_wait and len(si.on_wait) > 1:
                waits = list(si.on_wait)
                for w in waits[:-1]:
                    uid += 1
                    nop = mybir.InstNoOp(
                        name=f"waitsplit_{uid}",
                        ins=[], outs=[],
                        text_hint="wait_split", bass_nofuse=True,
                    )
                    nop.engine = inst.engine
                    nop.sync_info = mybir.SyncInfo(on_wait=[w], on_update=[])
                    nc.register_instruction(nop, overwrite=True)
                    out.append(nop)
                si.on_wait = waits[-1:]
            out.append(inst)
        bb.instructions = out


def _build_launch1():
    nc = bass.Bass(num_swdge_queues=4)
    xp = nc.dram_tensor("xp", [CIN, 32, WP], F32, kind="ExternalInput")
    w5af = nc.dram_tensor("w5af", [9, 4, 16, 128, 128], F32, kind="ExternalInput")
    shift1c = nc.dram_tensor("shift1c", [128, 4], F32, kind="ExternalInput")
    wqkc = nc.dram_tensor("wqkc", [128, 4, 128], F32, kind="ExternalInput")
    wvc = nc.dram_tensor("wvc", [128, 4, 512], F32, kind="ExternalInput")
    f1o = nc.dram_tensor("f1o", [128, 4, NH], F32, kind="ExternalOutput")
    qko = nc.dram_tensor("qko", [128, NH], F32, kind="ExternalOutput")
    vto = nc.dram_tensor("vto", [NH, 512], F32, kind="ExternalOutput")

    xp_r = xp.rearrange("(ck p) r c -> p ck r c", p=128)

    with tile.TileContext(nc) as tc:
        with nc.allow_low_precision(reason="f32r compute"):
            with (
                tc.tile_pool(name="fpool", bufs=1) as fpool,
                tc.tile_pool(name="spool", bufs=1) as spool,
                tc.tile_pool(name="ps", bufs=1, space="PSUM") as ps,
            ):
                sh_sb = spool.tile([128, 4], F32, tag="sh")
                nc.sync.dma_start(sh_sb[:], shift1c[:])
                f1_sb = fpool.tile([128, 4, NH], F32R)

                # ---- conv5a (+BN fold +ReLU) ----
                with tc.tile_pool(name="xpool", bufs=1) as xpool, \
                     tc.tile_pool(name="wpool", bufs=2) as wpool:
                    x_sb = xpool.tile([128, 16, 32, WP], F32R)
                    nc.gpsimd.dma_start(x_sb[:], xp_r)
                    for cc in range(4):
                        pf = [
                            ps.tile([128, 8 if nb < 3 else 6, 60], F32,
                                    tag=f"pf{nb}", name=f"pf{nb}")
                            for nb in range(4)
                        ]
                        for ti, (dy, dx) in enumerate(TAPS):
                            wt = wpool.tile([128, 16, 128], F32R, tag="wtap")
                            nc.gpsimd.dma_start(
                                wt[:], w5af[ti, cc].rearrange("ck p m -> p ck m")
                            )
                            for ck in range(16):
                                for nb in range(4):
                                    rows = 8 if nb < 3 else 6
                                    r0 = nb * 8 + 1 + dy
                                    nc.tensor.matmul(
                                        pf[nb][:],
                                        wt[:, ck, :],
                                        x_sb[:, ck, r0:r0 + rows, dx + 1:dx + 61],
                                        start=(ti == 0 and ck == 0),
                                        stop=(ti == 8 and ck == 15),
                                    )
                        for nb in range(4):
                            rows = 8 if nb < 3 else 6
                            dst = f1_sb[:, cc, nb * 480:nb * 480 + rows * 60]
                            nc.scalar.activation(
                                dst.rearrange("p (r c) -> p r c", c=60),
                                pf[nb][:],
                                AF.Relu,
                                bias=sh_sb[:, cc:cc + 1],
                            )
                            nc.sync.dma_start(
                                f1o[:, cc, nb * 480:nb * 480 + rows * 60],
                                dst.bitcast(F32),
                            )

                # ---- q/k and v^T 1x1 ----
                with tc.tile_pool(name="cpool", bufs=3) as cpool:
                    wqk_sb = cpool.tile([128, 4, 128], F32R, tag="wqk")
                    nc.gpsimd.dma_start(wqk_sb[:], wqkc[:])
                    for nb in range(4):
                        pq = ps.tile([128, 450], F32, tag="pqk")
                        for ck in range(4):
                            nc.tensor.matmul(
                                pq[:],
                                wqk_sb[:, ck, :],
                                f1_sb[:, ck, nb * 450:(nb + 1) * 450],
                                start=(ck == 0),
                                stop=(ck == 3),
                            )
                        cp = cpool.tile([128, 450], F32, tag="qkcp")
                        nc.vector.tensor_copy(cp[:], pq[:])
                        nc.sync.dma_start(qko[:, nb * 450:(nb + 1) * 450], cp[:])

                    wv_sb = cpool.tile([128, 4, 512], F32R, tag="wv")
                    nc.gpsimd.dma_start(wv_sb[:], wvc[:])
                    for jt in range(15):
                        pv = ps.tile([120, 512], F32, tag="pvt")
                        for ck in range(4):
                            nc.tensor.matmul(
                                pv[:],
                                f1_sb[:, ck, jt * 120:(jt + 1) * 120],
                                wv_sb[:, ck, :],
                                start=(ck == 0),
                                stop=(ck == 3),
                            )
                        cv = cpool.tile([120, 512], F32, tag="vtcp")
                        nc.vector.tensor_copy(cv[:], pv[:])
                        nc.sync.dma_start(vto[jt * 120:(jt + 1) * 120, :], cv[:])

    _split_multiwaits(nc)
    return nc


def _build_launch2():
    nc = bass.Bass(num_swdge_queues=4)
    qt = nc.dram_tensor("qt", [128, NI], F32, kind="ExternalInput")
    kt = nc.dram_tensor("kt", [128, NJ], F32, kind="ExternalInput")
    vt2 = nc.dram_tensor("vt2", [128, NJC, 512], F32, kind="ExternalInput")
    fsa = nc.dram_tensor("fsa", [128, 4, WIN * WP], F32, kind="ExternalInput")
    gsc = nc.dram_tensor("gsc", [1, NI], F32, kind="ExternalInput")
    onesr = nc.dram_tensor("onesr", [1, 128], F32, kind="ExternalInput")
    onesc = nc.dram_tensor("onesc", [128, 1], F32, kind="ExternalInput")
    w51f = nc.dram_tensor("w51f", [9, 4, 4, 128, 128], F32, kind="ExternalInput")
    shift2c = nc.dram_tensor("shift2c", [128, 4], F32, kind="ExternalInput")
    w6c = nc.dram_tensor("w6c", [128, 4, 64], F32, kind="ExternalInput")
    b6c = nc.dram_tensor("b6c", [64, 1], F32, kind="ExternalInput")
    lgo = nc.dram_tensor("lgo", [64, NH], F32, kind="ExternalOutput")

    with tile.TileContext(nc) as tc:
        with nc.allow_low_precision(reason="f32r compute"):
            with tc.tile_pool(name="fsap", bufs=1) as fsap:
                fsa_sb = fsap.tile([128, 4, WIN * WP], F32R)
                nc.gpsimd.dma_start(fsa_sb[:], fsa[:])
                fsa_r = fsa_sb.rearrange("p c (r x) -> p c r x", x=WP)

                # ================= attention =================
                with (
                    tc.tile_pool(name="att", bufs=1) as att,
                    tc.tile_pool(name="tmp", bufs=2) as tmpp,
                    tc.tile_pool(name="aps", bufs=1, space="PSUM") as aps,
                    tc.tile_pool(name="aps2", bufs=2, space="PSUM") as aps2,
                ):
                    qt_sb = att.tile([128, NI], F32)
                    nc.sync.dma_start(qt_sb[:], qt[:])
                    kt_sb = att.tile([128, NJ], F32)
                    nc.sync.dma_start(kt_sb[:], kt[:])
                    vt_sb = att.tile([128, NJC, 512], F32R)
                    nc.gpsimd.dma_start(vt_sb[:], vt2[:])
                    gsc_sb = att.tile([1, NI], F32)
                    nc.sync.dma_start(gsc_sb[:], gsc[:])
                    onesc_sb = att.tile([128, 1], F32R)
                    nc.gpsimd.dma_start(onesc_sb[:], onesc[:])
                    onesr_sb = att.tile([1, 128], F32)
                    nc.sync.dma_start(onesr_sb[:], onesr[:])

                    for ib in range(NIB):
                        isl = slice(ib * IBW, (ib + 1) * IBW)
                        mx = tmpp.tile([128, IBW], F32, tag="mx")
                        # pass A: fp32 similarity, running elementwise max
                        for jc in range(NJC):
                            ss = aps2.tile([128, IBW], F32, tag="ps_s")
                            nc.tensor.matmul(
                                ss[:],
                                kt_sb[:, jc * 128:(jc + 1) * 128],
                                qt_sb[:, isl],
                                start=True,
                                stop=True,
                            )
                            if jc == 0:
                                nc.vector.tensor_copy(mx[:], ss[:])
                            else:
                                nc.vector.tensor_max(mx[:], mx[:], ss[:])
                        m1 = tmpp.tile([1, IBW], F32, tag="m1")
                        nc.gpsimd.tensor_reduce(
                            m1[:], mx[:],
                            axis=mybir.AxisListType.C, op=mybir.AluOpType.max,
                        )
                        pmb = aps.tile([128, IBW], F32, tag="ps_mb")
                        nc.tensor.matmul(pmb[:], onesr_sb[:], m1[:], start=True, stop=True)
                        mxall = tmpp.tile([128, IBW], F32, tag="mxall")
                        nc.vector.tensor_copy(mxall[:], pmb[:])
                        # pass B: recompute sim, exp(sim - max), attn @ v
                        po = [
                            aps.tile([128, IBW], F32, tag=f"ps_o{c}", name=f"ps_o{c}")
                            for c in range(4)
                        ]
                        pd = aps.tile([1, IBW], F32, tag="ps_d")
                        for jc in range(NJC):
                            ss = aps2.tile([128, IBW], F32, tag="ps_s")
                            nc.tensor.matmul(
                                ss[:],
                                kt_sb[:, jc * 128:(jc + 1) * 128],
                                qt_sb[:, isl],
                                start=True,
                                stop=True,
                            )
                            tsub = tmpp.tile([128, IBW], F32, tag="tsub")
                            nc.vector.tensor_sub(tsub[:], ss[:], mxall[:])
                            et = tmpp.tile([128, IBW], F32R, tag="et")
                            nc.scalar.activation(et[:], tsub[:], AF.Exp)
                            for c in range(4):
                                nc.tensor.matmul(
                                    po[c][:],
                                    vt_sb[:, jc, c * 128:(c + 1) * 128],
                                    et[:],
                                    start=(jc == 0),
                                    stop=(jc == NJC - 1),
                                )
                            nc.tensor.matmul(
                                pd[:], onesc_sb[:], et[:],
                                start=(jc == 0), stop=(jc == NJC - 1),
                            )
                        rec = tmpp.tile([1, IBW], F32, tag="rec")
                        nc.vector.reciprocal(rec[:], pd[:])
                        gg = tmpp.tile([1, IBW], F32, tag="gg")
                        nc.vector.tensor_mul(gg[:], rec[:], gsc_sb[:, isl])
                        pgb = aps.tile([128, IBW], F32, tag="ps_mb")
                        nc.tensor.matmul(pgb[:], onesr_sb[:], gg[:], start=True, stop=True)
                        gg128 = tmpp.tile([128, IBW], F32, tag="gg128")
                        nc.vector.tensor_copy(gg128[:], pgb[:])
                        rsl = slice(ib * 8, (ib + 1) * 8)
                        for c in range(4):
                            t2 = tmpp.tile([128, 8, 60], F32, tag="t2")
                            nc.vector.tensor_mul(
                                t2[:],
                                po[c].rearrange("p (r c) -> p r c", c=60),
                                gg128.rearrange("p (r c) -> p r c", c=60),
                            )
                            nc.vector.tensor_add(
                                t2[:], t2[:],
                                fsa_r[:, c, rsl, 1:61].bitcast(F32),
                            )
                            nc.scalar.activation(
                                fsa_r[:, c, rsl, 1:61], t2[:], AF.Copy
                            )

                # ================= conv5c + conv6 =================
                with (
                    tc.tile_pool(name="cw", bufs=3) as cw,
                    tc.tile_pool(name="cc2", bufs=3) as cc2,
                    tc.tile_pool(name="scp", bufs=1) as scp,
                    tc.tile_pool(name="cps", bufs=1, space="PSUM") as cps,
                ):
                    sc_sb = scp.tile([128, 4, NH], F32R)
                    sh2_sb = cc2.tile([128, 4], F32, tag="sh2")
                    nc.sync.dma_start(sh2_sb[:], shift2c[:])
                    for cc in range(4):
                        pf = [
                            cps.tile([128, 8 if nb < 3 else 6, 60], F32,
                                     tag=f"pc{nb}", name=f"pc{nb}")
                            for nb in range(4)
                        ]
                        for ti, (dy, dx) in enumerate(TAPS):
                            wt = cw.tile([128, 4, 128], F32R, tag="w51t")
                            nc.gpsimd.dma_start(
                                wt[:], w51f[ti, cc].rearrange("ck p m -> p ck m")
                            )
                            for ck in range(4):
                                for nb in range(4):
                                    rows = 8 if nb < 3 else 6
                                    r0 = nb * 8 + 1 + dy
                                    nc.tensor.matmul(
                                        pf[nb][:],
                                        wt[:, ck, :],
                                        fsa_r[:, ck, r0:r0 + rows, dx + 1:dx + 61],
                                        start=(ti == 0 and ck == 0),
                                        stop=(ti == 8 and ck == 3),
                                    )
                        for nb in range(4):
                            rows = 8 if nb < 3 else 6
                            dst = sc_sb[:, cc, nb * 480:nb * 480 + rows * 60]
                            nc.scalar.activation(
                                dst.rearrange("p (r c) -> p r c", c=60),
                                pf[nb][:],
                                AF.Relu,
                                bias=sh2_sb[:, cc:cc + 1],
                            )

                    w6_sb = cc2.tile([128, 4, 64], F32R, tag="w6")
                    nc.gpsimd.dma_start(w6_sb[:], w6c[:])
                    b6_sb = cc2.tile([64, 1], F32, tag="b6")
                    nc.sync.dma_start(b6_sb[:], b6c[:])
                    for nb in range(4):
                        rows = 8 if nb < 3 else 6
                        pl = cps.tile([64, 480], F32, tag="pl")
                        for ck in range(4):
                            nc.tensor.matmul(
                                pl[:, :rows * 60],
                                w6_sb[:, ck, :],
                                sc_sb[:, ck, nb * 480:nb * 480 + rows * 60],
                                start=(ck == 0),
                                stop=(ck == 3),
                            )
                        lg = cc2.tile([64, 480], F32, tag="lg")
                        nc.vector.tensor_add(
                            lg[:, :rows * 60],
                            pl[:, :rows * 60],
                            b6_sb[:, 0:1].to_broadcast([64, rows * 60]),
                        )
                        nc.sync.dma_start(
                            lgo[:, nb * 480:nb * 480 + rows * 60], lg[:, :rows * 60]
                        )

    _split_multiwaits(nc)
    return nc


_NC_CACHE = {}


def _get_nc(which):
    if which not in _NC_CACHE:
        _NC_CACHE[which] = _build_launch1() if which == 1 else _build_launch2()
    return _NC_CACHE[which]


def kernel(**inputs):
    inp = {k: np.asarray(v, dtype=np.float32) for k, v in inputs.items()}
    x, dep = inp["x"], inp["dep"]
    lamb = float(inp["lamb1"][0])
    gamma = float(inp["gamma"][0])

    # ---- fold BN into conv weights (inference) ----
    s1 = inp["bn1_w"] / np.sqrt(inp["bn1_v"] + EPS)
    sh1 = inp["bn1_b"] - inp["bn1_m"] * s1
    w5a_f = inp["w5a"] * s1[:, None, None, None]          # [512, 2048, 3, 3]
    s2 = inp["bn2_w"] / np.sqrt(inp["bn2_v"] + EPS)
    sh2 = inp["bn2_b"] - inp["bn2_m"] * s2
    w51_f = inp["w51"] * s2[:, None, None, None]          # [512, 512, 3, 3]

    # w5af[tap, co_ck, ci_ck, p, m] = w5a_f[co_ck*128+m, ci_ck*128+p, dy+1, dx+1]
    w5af = np.empty((9, 4, 16, 128, 128), np.float32)
    w51f = np.empty((9, 4, 4, 128, 128), np.float32)
    for ti, (dy, dx) in enumerate(TAPS):
        wt = w5a_f[:, :, dy + 1, dx + 1]                   # [co, ci]
        w5af[ti] = wt.reshape(4, 128, 16, 128).transpose(0, 2, 3, 1)
        wt2 = w51_f[:, :, dy + 1, dx + 1]
        w51f[ti] = wt2.reshape(4, 128, 4, 128).transpose(0, 2, 3, 1)
    shift1c = np.ascontiguousarray(sh1.reshape(4, 128).T)
    shift2c = np.ascontiguousarray(sh2.reshape(4, 128).T)

    wq2 = inp["wq"][:, :, 0, 0]                            # [64, 512]
    wk2 = inp["wk"][:, :, 0, 0]
    wqk = np.concatenate([wq2, wk2], axis=0)               # [128, 512]
    wqkc = np.ascontiguousarray(wqk.reshape(128, 4, 128).transpose(2, 1, 0))
    wv2 = inp["wv"][:, :, 0, 0]                            # [512out, 512in]
    wvc = np.ascontiguousarray(wv2.reshape(512, 4, 128).transpose(2, 1, 0))
    w6c = np.zeros((128, 4, 64), np.float32)
    w6c[:, :, :COUT] = inp["w6"][:, :, 0, 0].reshape(COUT, 4, 128).transpose(2, 1, 0)
    b6c = np.zeros((64, 1), np.float32)
    b6c[:COUT, 0] = inp["b6"]

    # ---- launch 1 ----
    in_maps1 = []
    for c in range(8):
        b, h = c // 2, c % 2
        r0 = h * HH
        xpd = np.zeros((CIN, 32, WP), np.float32)
        lo, hi = max(0, r0 - 1), min(H, r0 + 31)
        xpd[:, lo - (r0 - 1):hi - (r0 - 1), 1:61] = x[b, :, lo:hi, :]
        in_maps1.append({
            "xp": xpd, "w5af": w5af, "shift1c": shift1c,
            "wqkc": wqkc, "wvc": wvc,
        })
    res1 = run_bass_kernel_spmd(_get_nc(1), in_maps1, core_ids=list(range(8)))

    # ---- host assembly between launches ----
    qk_full = np.empty((B, 128, H * W), np.float32)
    vt_full = np.empty((B, H * W, 512), np.float32)
    f1_full = np.empty((B, 128, 4, H * W), np.float32)
    for c in range(8):
        b, h = c // 2, c % 2
        jsl = slice(h * NH, (h + 1) * NH)
        r = res1.results[c]
        qk_full[b][:, jsl] = r["qko"]
        vt_full[b][jsl, :] = r["vto"]
        f1_full[b][:, :, jsl] = r["f1o"]
    qk_full[:, :CQ, :] += inp["bq"][None, :, None]
    qk_full[:, CQ:, :] += inp["bk"][None, :, None]
    vt_full += inp["bv"][None, None, :]

    dpf = dep.reshape(B, H * W)
    in_maps2 = []
    for c in range(8):
        b, h = c // 2, c % 2
        r0 = h * HH
        # 32-row window r0-1 .. r0+30; rows outside the image stay zero
        qtw = np.zeros((128, NI), np.float32)
        fsaw = np.zeros((128, 4, WIN, WP), np.float32)
        gscw = np.zeros((1, NI), np.float32)
        lo, hi = max(0, r0 - 1), min(H, r0 + 31)
        wlo, whi = lo - (r0 - 1), hi - (r0 - 1)
        wsl = slice(wlo * W, whi * W)
        isl = slice(lo * W, hi * W)
        qtw[:CQ, wsl] = qk_full[b][:CQ, isl]
        di = dpf[b][isl]
        qtw[CQ, wsl] = 2.0 * lamb * di
        qtw[CQ + 1, wsl] = -lamb * di * di
        qtw[CQ + 2, wsl] = 1.0
        fsaw[:, :, wlo:whi, 1:61] = (
            f1_full[b][:, :, isl].reshape(128, 4, whi - wlo, W)
        )
        gscw[0, wsl] = gamma
        ktw = np.zeros((128, NJ), np.float32)
        ktw[:CQ, :H * W] = qk_full[b][CQ:, :]
        dj = dpf[b]
        ktw[CQ, :H * W] = dj
        ktw[CQ + 1, :H * W] = 1.0
        ktw[CQ + 2, :H * W] = -lamb * dj * dj
        ktw[CQ + 1, H * W:] = 1.0
        ktw[CQ + 2, H * W:] = NEG
        vtp = np.zeros((NJ, 512), np.float32)
        vtp[:H * W] = vt_full[b]
        vt2 = np.ascontiguousarray(
            vtp.reshape(NJC, 128, 512).transpose(1, 0, 2)
        )
        in_maps2.append({
            "qt": qtw, "kt": ktw, "vt2": vt2,
            "fsa": fsaw.reshape(128, 4, WIN * WP), "gsc": gscw,
            "onesr": np.ones((1, 128), np.float32),
            "onesc": np.ones((128, 1), np.float32),
            "w51f": w51f, "shift2c": shift2c, "w6c": w6c, "b6c": b6c,
        })
    res2 = run_bass_kernel_spmd(_get_nc(2), in_maps2, core_ids=list(range(8)))

    logits = np.empty((B, COUT, H, W), np.float32)
    for c in range(8):
        b, h = c // 2, c % 2
        lg = res2.results[c]["lgo"]
        logits[b, :, h * HH:(h + 1) * HH, :] = lg[:COUT].reshape(COUT, HH, W)
    return (logits,)

